# revision 5
# baseline (speedup 1.0000x reference)
"""Trainium2 Bass kernel for KnowledgeDistillationGeometricJSLoss.

Full inputs: stu_corner, tea_corner [8388608, 4] fp32. Output: scalar fp32 mean loss.

Math (per row, per component c in {x,y}; comp x uses cols (0,2)=(l,r), y uses (1,3)=(t,b)):
  x1 = ln(l_s*r_s), x2 = ln(l_t*r_t)            # = 2*means
  A = x1^2 + 4e-6,  B = x2^2 + 4e-6             # = 4*cov diag
  u = A+B, w = A*B, h = u^2/w
  T_c = h/4 - 0.5*ln(h) + ln2 + 0.25*d^2*(h-2)/u   where d = x2-x1
  js  = 0.5*(T_x + T_y - 2)
  loss = 1 - 1/(1+js^2);  output = mean(loss) = (N - sum r)/N, r = 1/(1+js^2)

Only ln/exp/square transcendentals -> single ACT table set (natural_log_exp_and_others).
Shard N over 8 cores; per core stream 8 tiles of [128 partitions x 1024 rows x 4 cols];
per-tile partial sums of r ride activation accum_out into acc[128, 8]; host sums in f64.
"""
import math
from contextlib import ExitStack

import numpy as np

import concourse.bacc as bacc
import concourse.tile as tile
from concourse import mybir
from concourse.bass_utils import run_bass_kernel_spmd

N_FULL = 8388608
N_CORES = 8
R = N_FULL // N_CORES          # 1048576 rows per core
P = 128
ROWS_PP = R // P               # 8192 rows per partition
F = 1024                       # rows per partition per tile
NT = ROWS_PP // F              # 8 tiles
FP32 = mybir.dt.float32
LN2 = float(math.log(2.0))
LN4 = float(math.log(4.0))

_CACHED_NC = None


def _register_const(nc, value: float):
    t = nc.alloc_sbuf_tensor(f"const-f32-user-{value}", [128, 1], FP32)
    nc.gpsimd.memset(t.ap(), value)
    nc.const_aps.aps[(FP32, value)] = t.ap()


def _build(repeat: int = 1):
    nc = bacc.Bacc("TRN2", target_bir_lowering=False, debug=False)
    _register_const(nc, -LN4)
    nc.all_engine_barrier()
    stu = nc.dram_tensor("stu", [R, 4], FP32, kind="ExternalInput").ap()
    tea = nc.dram_tensor("tea", [R, 4], FP32, kind="ExternalInput").ap()
    acc_d = nc.dram_tensor("acc", [P, NT], FP32, kind="ExternalOutput").ap()

    stu_v = stu.rearrange("(p n) c -> p n c", p=P)   # [128, 8192, 4]
    tea_v = tea.rearrange("(p n) c -> p n c", p=P)

    AF = mybir.ActivationFunctionType
    with tile.TileContext(nc) as tc, ExitStack() as ctx:
        inp = ctx.enter_context(tc.tile_pool(name="inp", bufs=2))
        pp = ctx.enter_context(tc.tile_pool(name="pp", bufs=2))
        mid = ctx.enter_context(tc.tile_pool(name="mid", bufs=2))
        accp = ctx.enter_context(tc.tile_pool(name="accp", bufs=1))

        acc_sb = accp.tile([P, NT], FP32)

        def body():
            for t in range(NT):
                stu_t = inp.tile([P, F * 4], FP32, tag="stu_t")
                nc.sync.dma_start(stu_t[:], stu_v[:, t * F:(t + 1) * F, :])
                tea_t = inp.tile([P, F * 4], FP32, tag="tea_t")
                nc.sync.dma_start(tea_t[:], tea_v[:, t * F:(t + 1) * F, :])
                stu4 = stu_t[:].rearrange("p (n c) -> p n c", c=4)
                tea4 = tea_t[:].rearrange("p (n c) -> p n c", c=4)

                # P tile: [128, 2, F, 2]  (dim1: 0=stu, 1=tea; dim3: component)
                Pt = pp.tile([P, 4 * F], FP32, tag="Pt")
                P4 = Pt[:].rearrange("p (s n c) -> p s n c", s=2, c=2)
                nc.vector.tensor_mul(P4[:, 0], stu4[:, :, 0:2], stu4[:, :, 2:4])
                nc.vector.tensor_mul(P4[:, 1], tea4[:, :, 0:2], tea4[:, :, 2:4])
                # L = ln(P) in place; x1 = L[:,0], x2 = L[:,1]  (each [128, F, 2])
                nc.scalar.activation(Pt[:], Pt[:], AF.Ln)
                x1 = P4[:, 0].rearrange("p n c -> p (n c)")
                x2 = P4[:, 1].rearrange("p n c -> p (n c)")

                # d^2 (sub on DVE, square on ACT)
                d_t = mid.tile([P, 2 * F], FP32, tag="d_t")
                nc.vector.tensor_sub(d_t[:], x2, x1)
                nc.scalar.activation(d_t[:], d_t[:], AF.Square)
                # A = x1^2 + eps (ACT square then scalar add), B likewise
                A_t = mid.tile([P, 2 * F], FP32, tag="A_t")
                nc.scalar.activation(A_t[:], x1, AF.Square)
                nc.vector.tensor_scalar_add(A_t[:], A_t[:], 4e-6)
                B_t = mid.tile([P, 2 * F], FP32, tag="B_t")
                nc.scalar.activation(B_t[:], x2, AF.Square)
                nc.vector.tensor_scalar_add(B_t[:], B_t[:], 4e-6)
                # sAB = A+B ; pq = A*B (into A)
                sAB = mid.tile([P, 2 * F], FP32, tag="sAB")
                nc.vector.tensor_add(sAB[:], A_t[:], B_t[:])
                nc.vector.tensor_mul(A_t[:], A_t[:], B_t[:])
                # Lu = ln(sAB) in place ; Lw = ln(pq) in place (over A)
                nc.scalar.activation(sAB[:], sAB[:], AF.Ln)
                nc.scalar.activation(A_t[:], A_t[:], AF.Ln)
                # zh2 = (Lw*0.5) - Lu   (fused stt, in place over A)
                nc.vector.scalar_tensor_tensor(
                    A_t[:], A_t[:], 0.5, sAB[:],
                    op0=mybir.AluOpType.mult, op1=mybir.AluOpType.subtract,
                )
                # h4 = exp(-2*zh2 - ln4) ; ru = exp(-Lu) in place over sAB
                h4 = mid.tile([P, 2 * F], FP32, tag="h4")
                nc.scalar.activation(h4[:], A_t[:], AF.Exp, bias=-LN4, scale=-2.0)
                nc.scalar.activation(sAB[:], sAB[:], AF.Exp, scale=-1.0)
                # m1 = (h4 - 0.5)*d^2 (fused stt, into d) ; m2 = m1*ru (into d)
                nc.vector.scalar_tensor_tensor(
                    d_t[:], h4[:], 0.5, d_t[:],
                    op0=mybir.AluOpType.subtract, op1=mybir.AluOpType.mult,
                )
                nc.vector.tensor_mul(d_t[:], d_t[:], sAB[:])
                # T = h4 + zh2 + m2  (into A) - offloaded to gpsimd (DVE is the
                # bottleneck engine; gpsimd is otherwise idle)
                nc.gpsimd.tensor_add(A_t[:], h4[:], A_t[:])
                nc.gpsimd.tensor_add(A_t[:], A_t[:], d_t[:])
                # S = T_x + T_y ; js = 0.5*S + (ln2-1) ; jsq = js^2
                T2 = A_t[:].rearrange("p (n c) -> p n c", c=2)
                S_t = mid.tile([P, F], FP32, tag="S_t")
                nc.vector.tensor_add(S_t[:], T2[:, :, 0], T2[:, :, 1])
                nc.vector.tensor_scalar(
                    S_t[:], S_t[:], 0.5, LN2 - 1.0,
                    mybir.AluOpType.mult, mybir.AluOpType.add,
                )
                nc.vector.tensor_mul(S_t[:], S_t[:], S_t[:])
                # r = exp(-ln(1+jsq)); partial sum rides accum_out
                nc.scalar.activation(S_t[:], S_t[:], AF.Ln, bias=1.0)
                nc.scalar.activation(
                    S_t[:], S_t[:], AF.Exp, scale=-1.0,
                    accum_out=acc_sb[:, t:t + 1],
                )

        if repeat == 1:
            body()
        else:
            with tc.For_i(0, repeat, 1):
                body()

        nc.sync.dma_start(acc_d[:], acc_sb[:])
    nc.compile()
    return nc


def _get_nc():
    global _CACHED_NC
    if _CACHED_NC is None:
        _CACHED_NC = _build(1)
    return _CACHED_NC


def kernel(stu_corner: np.ndarray, tea_corner: np.ndarray) -> np.ndarray:
    nc = _get_nc()
    stu8 = np.ascontiguousarray(stu_corner.reshape(N_CORES, R, 4))
    tea8 = np.ascontiguousarray(tea_corner.reshape(N_CORES, R, 4))
    in_maps = [{"stu": stu8[i], "tea": tea8[i]} for i in range(N_CORES)]
    res = run_bass_kernel_spmd(nc, in_maps, list(range(N_CORES)))
    total_r = 0.0
    for i in range(N_CORES):
        total_r += res.results[i]["acc"].astype(np.float64).sum()
    loss = (N_FULL - total_r) / N_FULL
    return np.float32(loss)


if __name__ == "__main__":
    rng = np.random.default_rng(0)
    stu = (rng.random((N_FULL, 4), dtype=np.float32) * 256.0 + 1e-3)
    tea = (rng.random((N_FULL, 4), dtype=np.float32) * 256.0 + 1e-3)
    print("loss:", kernel(stu, tea))
